# revision 7
# baseline (speedup 1.0000x reference)
"""Trainium2 Bass kernel for nn_AttentionBlock_86715389706345.

Math (exact reduction of the reference):
  The reference multiplies attn by ones(T,T) and softmaxes over the query
  axis, so per (b,h): rowsum[t] = q[t]·ksum/sqrt(DH), ksum = sum_s k[s],
  p = softmax_t(rowsum), head_out[t,:] = p[t]*vsum, vsum = sum_t v[t].
  Further ksum = xsum @ Wk[h], vsum = xsum @ Wv[h] (xsum = sum_t x[t]) and
  rowsum[t,h] = x[t]·U[h]/sqrt(DH) with U[h] = Wq[h] @ ksum[h].
  Then x1 = LN(x + out), y = LN(x1 + relu(x1@W1)@W2).

Sharding: data-parallel — batch b on core b (B == n_cores == 8), weights
replicated, no collectives.

Precision: prep chain (xsum/ksum/vsum/U/rowsum) via fp32r matmuls (f32
storage, reduced-precision PE mode), FFN matmuls bf16. Validated ~<1e-3
end-to-end error in simulation.
"""
import sys
sys.path.insert(0, '/opt/trn_rl_repo')
import numpy as np

import concourse.bass as bass
import concourse.tile as tile
import concourse.mybir as mybir
from concourse.bass_utils import run_bass_kernel_spmd
from concourse.masks import make_identity

F32 = mybir.dt.float32
BF16 = mybir.dt.bfloat16
F32R = mybir.dt.float32r
AF = mybir.ActivationFunctionType
OP = mybir.AluOpType
AX = mybir.AxisListType

B, T, D, H = 8, 1024, 1024, 16
DH = D // H
EPS = 1e-5
P = 128
NT = T // P   # 8 row tiles
NC = D // P   # 8 col tiles
N_CORES = 8
RSCALE = float(1.0 / np.sqrt(DH))


def _expand_ap(ap, reps):
    """Append a step-0 broadcast dim of size `reps` to an AP."""
    return bass.AP(tensor=ap.tensor, offset=ap.offset,
                   ap=[list(dd) for dd in ap.ap] + [[0, reps]])


def _w_ctile(wdram, j):
    """AP view of W (H, D, DH) as [c-part 128, h 16, d 64] for c-tile j."""
    base = wdram[:]
    return bass.AP(tensor=base.tensor, offset=j * P * DH,
                   ap=[[DH, P], [D * DH, H], [1, DH]])


def _split_waits(nc):
    """This container's walrus accepts ONE sync wait per instruction; Tile
    emits 2-3. Hoist extras onto single-wait NoOps on the same engine placed
    immediately before (engines execute block-order)."""
    k = 0
    for f in nc.m.functions:
        for bb in f.blocks:
            out = []
            changed = False
            for ins in bb.instructions:
                si = getattr(ins, "sync_info", None)
                if si is not None and len(si.on_wait) > 1:
                    for w in si.on_wait[:-1]:
                        nop = mybir.InstNoOp(name=f"I-waitfix-{k}")
                        k += 1
                        nop.engine = ins.engine
                        nop.sync_info = mybir.SyncInfo(on_wait=[w], on_update=[])
                        out.append(nop)
                    ins.sync_info = mybir.SyncInfo(
                        on_wait=[si.on_wait[-1]], on_update=list(si.on_update))
                    changed = True
                out.append(ins)
            if changed:
                bb.instructions = out
    return k


def build():
    nc = bass.Bass()
    x = nc.dram_tensor("x", [T, D], F32, kind="ExternalInput")
    Wq = nc.dram_tensor("Wq", [H, D, DH], F32, kind="ExternalInput")
    Wk = nc.dram_tensor("Wk", [H, D, DH], F32, kind="ExternalInput")
    Wv = nc.dram_tensor("Wv", [H, D, DH], F32, kind="ExternalInput")
    W1 = nc.dram_tensor("W1", [D, D], F32, kind="ExternalInput")
    W2 = nc.dram_tensor("W2", [D, D], F32, kind="ExternalInput")
    g1 = nc.dram_tensor("g1", [D], F32, kind="ExternalInput")
    b1 = nc.dram_tensor("b1", [D], F32, kind="ExternalInput")
    g2 = nc.dram_tensor("g2", [D], F32, kind="ExternalInput")
    b2 = nc.dram_tensor("b2", [D], F32, kind="ExternalInput")
    out = nc.dram_tensor("out", [T, D], F32, kind="ExternalOutput")

    xr = x.rearrange("(i p) d -> i p d", p=P)
    outr = out.rearrange("(i p) d -> i p d", p=P)
    w1r = W1.rearrange("(j p) d -> j p d", p=P)
    w2r = W2.rearrange("(j p) d -> j p d", p=P)

    with tile.TileContext(nc) as tc:
        with tc.tile_pool(name="px", bufs=NT) as px, \
             tc.tile_pool(name="pxT", bufs=NC) as pxT, \
             tc.tile_pool(name="pw1", bufs=NC) as pw1, \
             tc.tile_pool(name="pw2", bufs=NC) as pw2, \
             tc.tile_pool(name="px1T", bufs=NC) as px1T, \
             tc.tile_pool(name="pwst", bufs=2) as pwst, \
             tc.tile_pool(name="pbc", bufs=5) as pbc, \
             tc.tile_pool(name="ptmp", bufs=3) as ptmp, \
             tc.tile_pool(name="pz2", bufs=2) as pz2, \
             tc.tile_pool(name="psmall", bufs=8) as psmall, \
             tc.tile_pool(name="prow", bufs=1) as prow, \
             tc.tile_pool(name="pconst", bufs=1) as pconst, \
             tc.tile_pool(name="ppt", bufs=2, space="PSUM") as ppt, \
             tc.tile_pool(name="ppm", bufs=2, space="PSUM") as ppm, \
             tc.tile_pool(name="ppa", bufs=1, space="PSUM") as ppa:

            ident = pconst.tile([P, P], F32)
            make_identity(nc, ident)
            ones_f = pconst.tile([1, P], F32)
            nc.vector.memset(ones_f[:], 1.0)
            ones_col = pconst.tile([1, P], F32R)
            nc.vector.tensor_copy(ones_col[:], ones_f[:])
            eps_t = pconst.tile([P, 1], F32)
            nc.vector.memset(eps_t[:], EPS)

            # ---- load x ----
            xs = []
            for i in range(NT):
                t = px.tile([P, D], F32, tag="x")
                nc.sync.dma_start(t[:], xr[i])
                xs.append(t)

            # ---- W1/W2 with DMA cast f32->bf16 (SWDGE) ----
            w1b, w2b = [], []
            for j in range(NC):
                t1 = pw1.tile([P, D], BF16, tag="w1")
                nc.gpsimd.dma_start(t1[:], w1r[j])
                w1b.append(t1)
            for j in range(NC):
                t2 = pw2.tile([P, D], BF16, tag="w2")
                nc.gpsimd.dma_start(t2[:], w2r[j])
                w2b.append(t2)

            # ---- g/b broadcast tiles ----
            def bcast_vec(v):
                tt = pbc.tile([P, D], F32, tag="bc")
                src = bass.AP(tensor=v[:].tensor, offset=0, ap=[[0, P], [1, D]])
                nc.gpsimd.dma_start(tt[:], src)
                return tt
            g1bc = bcast_vec(g1)
            b1bc = bcast_vec(b1)
            g2bc = bcast_vec(g2)
            b2bc = bcast_vec(b2)

            # ---- acc = sum_i x_i (tree over DVE/GPSIMD) ----
            a01 = ptmp.tile([P, D], F32, tag="acc")
            nc.vector.tensor_add(a01[:], xs[0][:], xs[1][:])
            a23 = ptmp.tile([P, D], F32, tag="acc")
            nc.gpsimd.tensor_add(a23[:], xs[2][:], xs[3][:])
            a03 = ptmp.tile([P, D], F32, tag="acc")
            nc.vector.tensor_add(a03[:], a01[:], a23[:])
            a45 = ptmp.tile([P, D], F32, tag="acc")
            nc.vector.tensor_add(a45[:], xs[4][:], xs[5][:])
            a67 = ptmp.tile([P, D], F32, tag="acc")
            nc.gpsimd.tensor_add(a67[:], xs[6][:], xs[7][:])
            nc.vector.tensor_add(a45[:], a45[:], a67[:])
            nc.vector.tensor_add(a45[:], a03[:], a45[:])
            acc = a45

            # ---- xsumT (c-part) via transposes + segmented reduce ----
            xsumT_f = psmall.tile([P, NC], F32, tag="xsumTf")
            for g in range(2):
                ptr = ppt.tile([P, 512], F32, tag="tr")
                for u in range(4):
                    j = g * 4 + u
                    nc.tensor.transpose(ptr[:, u * P:(u + 1) * P],
                                        acc[:, j * P:(j + 1) * P], ident[:])
                nc.vector.tensor_reduce(
                    xsumT_f[:, g * 4:(g + 1) * 4],
                    ptr[:].rearrange("p (u q) -> p u q", u=4),
                    axis=AX.X, op=OP.add)
            xsumT = psmall.tile([P, NC], F32R, tag="xsumT")
            nc.vector.tensor_copy(xsumT[:], xsumT_f[:])

            # ---- xT tiles (c-part, fp32r) via PE transposes ----
            xT = []
            for j in range(NC):
                tj = pxT.tile([P, T], F32R, tag="xT")
                for g in range(2):
                    ptr = ppt.tile([P, 512], F32, tag="tr")
                    for u in range(4):
                        i = g * 4 + u
                        nc.tensor.transpose(ptr[:, u * P:(u + 1) * P],
                                            xs[i][:, j * P:(j + 1) * P], ident[:])
                    nc.vector.tensor_copy(tj[:, g * 512:(g + 1) * 512], ptr[:])
                xT.append(tj)

            # ---- ksum then vsum rows (M=1 fp32r matmuls, streamed W) ----
            def sum_row(wdram, name):
                ps_row = ppa.tile([H, T], F32, tag="arow")  # use [0:1, :]
                for j in range(NC):
                    wj = pwst.tile([P, D], F32R, tag="wst")
                    nc.sync.dma_start(
                        wj[:].rearrange("p (h d) -> p h d", h=H),
                        _w_ctile(wdram, j).bitcast(F32R))
                    lhs = xsumT[:, j:j + 1]
                    for h2 in range(2):
                        sl = slice(h2 * 512, (h2 + 1) * 512)
                        nc.tensor.matmul(ps_row[0:1, sl], lhs, wj[:, sl],
                                         start=(j == 0), stop=(j == NC - 1))
                row = prow.tile([1, D], F32R, tag=name)
                nc.vector.tensor_copy(row[:], ps_row[0:1, :])
                return row
            ksum_row = sum_row(Wk, "krow")

            # ---- ksum broadcast (128, D) via K=1 matmul ----
            kb_ps = ppm.tile([P, D], F32, tag="mm")
            for h2 in range(2):
                sl = slice(h2 * 512, (h2 + 1) * 512)
                nc.tensor.matmul(kb_ps[:, sl], ones_col[:], ksum_row[:, sl],
                                 start=True, stop=True)
            ksum_b = pbc.tile([P, D], F32, tag="bc")
            nc.vector.tensor_copy(ksum_b[:], kb_ps[:])

            vsum_row = sum_row(Wv, "vrow")

            # ---- U (c-part, H) per j: DVE mul + segmented reduce ----
            UT = []
            for j in range(NC):
                wqj = pwst.tile([P, D], F32, tag="wst")
                nc.sync.dma_start(
                    wqj[:].rearrange("p (h d) -> p h d", h=H), _w_ctile(Wq, j))
                prod = ptmp.tile([P, D], F32, tag="acc")
                nc.vector.tensor_mul(prod[:], wqj[:], ksum_b[:])
                utf = psmall.tile([P, H], F32, tag="UTf")
                nc.vector.tensor_reduce(
                    utf[:], prod[:].rearrange("p (h d) -> p h d", h=H),
                    axis=AX.X, op=OP.add)
                utj = psmall.tile([P, H], F32R, tag="UT")
                nc.vector.tensor_copy(utj[:], utf[:])
                UT.append(utj)

            # ---- rowsumT (16, T) accumulated over j ----
            rs_ps = ppa.tile([H, T], F32, tag="arow")
            for j in range(NC):
                for h2 in range(2):
                    sl = slice(h2 * 512, (h2 + 1) * 512)
                    nc.tensor.matmul(rs_ps[:, sl], UT[j][:], xT[j][:, sl],
                                     start=(j == 0), stop=(j == NC - 1))

            # ---- softmax over t: p = exp(r/8 - max/8)/sum ----
            mx = psmall.tile([H, 1], F32, tag="mx")
            nc.vector.tensor_reduce(mx[:], rs_ps[:], axis=AX.X, op=OP.max)
            negmx = psmall.tile([H, 1], F32, tag="negmx")
            nc.scalar.mul(negmx[:], mx[:], -RSCALE)
            e_sb = prow.tile([H, T], F32, tag="esb")
            sumexp = psmall.tile([H, 1], F32, tag="sumexp")
            nc.scalar.activation(e_sb[:], rs_ps[:], AF.Exp,
                                 bias=negmx[:], scale=RSCALE,
                                 accum_out=sumexp[:])
            rec = psmall.tile([H, 1], F32, tag="rec")
            nc.vector.reciprocal(rec[:], sumexp[:])

            # rec (16,1) -> rec_row (1,16)
            rr_ps = ppt.tile([P, 512], F32, tag="tr")
            nc.tensor.transpose(rr_ps[0:1, 0:H], rec[:], ident[:H, :H])
            rec_row = prow.tile([1, H], F32, tag="recrow")
            nc.scalar.copy(rec_row[:], rr_ps[0:1, 0:H])

            # vsum_n = vsum_row * rec_row (head-broadcast along d)
            vsum_nf = prow.tile([1, D], F32, tag="vnf")
            nc.vector.tensor_tensor(
                out=vsum_nf[:].rearrange("p (h d) -> p h d", h=H),
                in0=vsum_row[:].bitcast(F32).rearrange("p (h d) -> p h d", h=H),
                in1=_expand_ap(rec_row[:], DH), op=OP.mult)
            vsum_n = prow.tile([1, D], F32R, tag="vn")
            nc.vector.tensor_copy(vsum_n[:], vsum_nf[:])

            # vsum broadcast (128, D)
            vb_ps = ppm.tile([P, D], F32, tag="mm")
            for h2 in range(2):
                sl = slice(h2 * 512, (h2 + 1) * 512)
                nc.tensor.matmul(vb_ps[:, sl], ones_col[:], vsum_n[:, sl],
                                 start=True, stop=True)
            vsum_b = pbc.tile([P, D], F32, tag="bc")
            nc.vector.tensor_copy(vsum_b[:], vb_ps[:])

            # e (16, T) -> eT_i (128, 16)
            eT = []
            for i in range(NT):
                et_ps = ppt.tile([P, 512], F32, tag="tr")
                nc.tensor.transpose(et_ps[:, 0:H],
                                    e_sb[:, i * P:(i + 1) * P], ident[:H, :H])
                ei = psmall.tile([P, H], F32, tag="eT")
                nc.scalar.copy(ei[:], et_ps[:, 0:H])
                eT.append(ei)

            # ---- LN1 per row tile ----
            BNS = nc.vector.BN_STATS_DIM
            BNA = nc.vector.BN_AGGR_DIM
            for i in range(NT):
                xi = xs[i]
                tmp = ptmp.tile([P, D], F32, tag="acc")
                nc.vector.tensor_tensor(
                    out=tmp[:].rearrange("p (h d) -> p h d", h=H),
                    in0=vsum_b[:].rearrange("p (h d) -> p h d", h=H),
                    in1=_expand_ap(eT[i][:], DH), op=OP.mult)
                nc.vector.tensor_add(xi[:], xi[:], tmp[:])  # z in place
                stats = psmall.tile([P, 2, BNS], F32, tag="stats")
                zr = xi[:].rearrange("p (g d) -> p g d", g=2)
                nc.vector.bn_stats(out=stats[:, 0, :], in_=zr[:, 0, :])
                nc.vector.bn_stats(out=stats[:, 1, :], in_=zr[:, 1, :])
                mv = psmall.tile([P, BNA], F32, tag="mv")
                nc.vector.bn_aggr(out=mv[:], in_=stats[:])
                s = psmall.tile([P, 1], F32, tag="s")
                nc.scalar.activation(s[:], mv[:, 1:2], AF.Sqrt, bias=eps_t[:])
                nc.vector.reciprocal(s[:], s[:])
                ms = psmall.tile([P, 1], F32, tag="ms")
                nc.vector.tensor_mul(ms[:], mv[:, 0:1], s[:])
                nc.scalar.activation(xi[:], xi[:], AF.Copy, bias=0.0, scale=s[:])
                nc.vector.scalar_tensor_tensor(
                    out=xi[:], in0=xi[:], scalar=ms[:], in1=g1bc[:],
                    op0=OP.subtract, op1=OP.mult)
                nc.gpsimd.tensor_add(xi[:], xi[:], b1bc[:])

            # ---- x1T (bf16) via PE transposes, batched by j ----
            x1T = []
            for j in range(NC):
                tj = px1T.tile([P, T], BF16, tag="x1T")
                for g in range(2):
                    ptr = ppt.tile([P, 512], F32, tag="tr")
                    for u in range(4):
                        i = g * 4 + u
                        nc.tensor.transpose(ptr[:, u * P:(u + 1) * P],
                                            xs[i][:, j * P:(j + 1) * P],
                                            ident[:])
                    nc.scalar.copy(tj[:, g * 512:(g + 1) * 512], ptr[:])
                x1T.append(tj)

            # ---- FFN mm1 + relu -> h1T (bf16), reusing xT pool slots ----
            h1T = []
            for f in range(NC):
                hp = ppm.tile([P, T], F32, tag="mm")
                for c in range(NC):
                    for h2 in range(2):
                        sl = slice(h2 * 512, (h2 + 1) * 512)
                        nc.tensor.matmul(hp[:, sl],
                                         w1b[c][:, f * P:(f + 1) * P],
                                         x1T[c][:, sl],
                                         start=(c == 0), stop=(c == NC - 1))
                hf = pxT.tile([P, T], BF16, tag="xT")
                nc.scalar.activation(hf[:], hp[:], AF.Relu)
                h1T.append(hf)

            # ---- FFN mm2 + residual + LN2 -> out ----
            for i in range(NT):
                fp = ppm.tile([P, D], F32, tag="mm")
                for f in range(NC):
                    for h2 in range(2):
                        sl = slice(h2 * 512, (h2 + 1) * 512)
                        nc.tensor.matmul(fp[:, sl],
                                         h1T[f][:, i * P:(i + 1) * P],
                                         w2b[f][:, sl],
                                         start=(f == 0), stop=(f == NC - 1))
                z2 = pz2.tile([P, D], F32, tag="z2")
                nc.vector.tensor_add(z2[:], fp[:], xs[i][:])
                stats2 = psmall.tile([P, 2, BNS], F32, tag="stats")
                z2r = z2[:].rearrange("p (g d) -> p g d", g=2)
                nc.vector.bn_stats(out=stats2[:, 0, :], in_=z2r[:, 0, :])
                nc.vector.bn_stats(out=stats2[:, 1, :], in_=z2r[:, 1, :])
                mv2 = psmall.tile([P, BNA], F32, tag="mv")
                nc.vector.bn_aggr(out=mv2[:], in_=stats2[:])
                s2 = psmall.tile([P, 1], F32, tag="s")
                nc.scalar.activation(s2[:], mv2[:, 1:2], AF.Sqrt, bias=eps_t[:])
                nc.vector.reciprocal(s2[:], s2[:])
                ms2 = psmall.tile([P, 1], F32, tag="ms")
                nc.vector.tensor_mul(ms2[:], mv2[:, 0:1], s2[:])
                nc.scalar.activation(z2[:], z2[:], AF.Copy, bias=0.0, scale=s2[:])
                nc.vector.scalar_tensor_tensor(
                    out=z2[:], in0=z2[:], scalar=ms2[:], in1=g2bc[:],
                    op0=OP.subtract, op1=OP.mult)
                nc.gpsimd.tensor_add(z2[:], z2[:], b2bc[:])
                nc.sync.dma_start(outr[i], z2[:])

    _split_waits(nc)
    return nc


_NC_CACHE = None


def kernel(x, Wq, Wk, Wv, W1, W2, g1, b1, g2, b2):
    global _NC_CACHE
    if _NC_CACHE is None:
        _NC_CACHE = build()
    nc = _NC_CACHE
    common = {"Wq": np.ascontiguousarray(Wq, np.float32),
              "Wk": np.ascontiguousarray(Wk, np.float32),
              "Wv": np.ascontiguousarray(Wv, np.float32),
              "W1": np.ascontiguousarray(W1, np.float32),
              "W2": np.ascontiguousarray(W2, np.float32),
              "g1": np.ascontiguousarray(g1, np.float32),
              "b1": np.ascontiguousarray(b1, np.float32),
              "g2": np.ascontiguousarray(g2, np.float32),
              "b2": np.ascontiguousarray(b2, np.float32)}
    in_maps = [dict(common, x=np.ascontiguousarray(np.asarray(x)[b], np.float32))
               for b in range(B)]
    res = run_bass_kernel_spmd(nc, in_maps, list(range(N_CORES)))
    return np.stack([res.results[b]["out"] for b in range(B)], axis=0)
